# revision 5
# baseline (speedup 1.0000x reference)
"""Trainium2 Bass kernel for nn_AnisotropicStack (ragged EMA tokenizer/detokenizer).

Self-contained: builds + compiles an 8-core SPMD Bass kernel (one batch row
per core), runs via bass_utils.run_bass_kernel_spmd, returns (output, new_state).

Algorithm per core (batch row):
  1. Host precomputes compaction indices (selected token positions), chunk
     probs, and expansion indices from the boolean mask (tiny int work).
  2. Device gathers the M=2048 selected x rows via indirect DMA into a
     (128 chunks x 16 steps) layout, runs the EMA scan:
        pass1: in-chunk scan (16 scalar_tensor_tensor steps, chunks on partitions)
        pass2: cross-chunk carry via log-space prefix + masked-exp matmul
        pass3: apply carry -> bf16 EMA table
  3. Table -> DRAM; scatter-expand = indirect row gather + residual add,
     streamed over 64 l-tiles; row 8192 of the output carries new_state.
"""
import sys
import types

for _p in ("/opt/trn_rl_repo",):
    if _p not in sys.path:
        sys.path.append(_p)

import numpy as np

B, L, D = 8, 8192, 1024
M = L // 4            # 2048 chunk slots
NCH, JW = 128, 16     # chunks on partitions, steps within chunk (NCH*JW == M)
NLT = L // 128        # 64 l-tiles for expansion
ZROW, SROW = M, M + 1 # table rows: zeros row, carried-state row

_CACHE = {}


def _build():
    import concourse.bacc as bacc
    import concourse.mybir as mybir
    import concourse.tile as tile
    from concourse import bass
    from concourse.bass import IndirectOffsetOnAxis

    f32 = mybir.dt.float32
    bf16 = mybir.dt.bfloat16
    i32 = mybir.dt.int32
    op = mybir.AluOpType

    nc = bacc.Bacc("TRN2", target_bir_lowering=False)

    x_d = nc.dram_tensor("x", (L, D), f32, kind="ExternalInput")
    res_d = nc.dram_tensor("res", (L, D), f32, kind="ExternalInput")
    state_d = nc.dram_tensor("state", (1, D), f32, kind="ExternalInput")
    selidx_d = nc.dram_tensor("selidx", (NCH, JW), i32, kind="ExternalInput")
    cp_d = nc.dram_tensor("cp", (NCH, JW), f32, kind="ExternalInput")
    cig_d = nc.dram_tensor("cig", (128, NLT + 1), i32, kind="ExternalInput")
    out_d = nc.dram_tensor("out", (L + 1, D), f32, kind="ExternalOutput")

    # constants
    k = np.arange(128)[:, None]
    m = np.arange(128)[None, :]
    lt_incl_np = (k <= m).astype(np.float32)           # [k, m] = k<=m
    lt_strict_np = (k < m).astype(np.float32)          # [k, m] = k<m
    mb_np = np.where(k < m, 0.0, -1e5).astype(np.float32)  # keep c'<c
    lt_incl_d = nc.inline_tensor(lt_incl_np, name="lt_incl")
    lt_strict_d = nc.inline_tensor(lt_strict_np, name="lt_strict")
    mb_d = nc.inline_tensor(mb_np, name="mb")

    with tile.TileContext(nc) as tc:
        with tc.tile_pool(name="persist", bufs=1) as pp, \
             tc.tile_pool(name="resp", bufs=8) as resp, \
             tc.tile_pool(name="gathp", bufs=4) as gathp, \
             tc.tile_pool(name="outp", bufs=4) as outp, \
             tc.tile_pool(name="psum", bufs=1, space="PSUM") as psp, \
             tc.tile_pool(name="dram", bufs=1, space="DRAM") as dp:

            table = dp.tile([M + 2, D], bf16)

            # ---- small loads ----
            selidx_t = pp.tile([NCH, JW], i32)
            nc.sync.dma_start(out=selidx_t[:], in_=selidx_d[:])
            cp_t = pp.tile([NCH, JW], f32)
            nc.sync.dma_start(out=cp_t[:], in_=cp_d[:])
            cig_t = pp.tile([128, NLT + 1], i32)
            nc.sync.dma_start(out=cig_t[:], in_=cig_d[:])
            state_t = pp.tile([1, D], f32)
            nc.sync.dma_start(out=state_t[:], in_=state_d[:])
            lti_t = pp.tile([128, 128], f32)
            nc.sync.dma_start(out=lti_t[:], in_=lt_incl_d[:])
            lts_t = pp.tile([128, 128], f32)
            nc.sync.dma_start(out=lts_t[:], in_=lt_strict_d[:])
            mb_t = pp.tile([128, 128], f32)
            nc.sync.dma_start(out=mb_t[:], in_=mb_d[:])

            # ---- gather selected x rows: CH[c, j*D:(j+1)*D] = x[sel[c*16+j]] ----
            CH = pp.tile([NCH, JW * D], f32)
            for j in range(JW):
                nc.gpsimd.indirect_dma_start(
                    out=CH[:, j * D:(j + 1) * D],
                    out_offset=None,
                    in_=x_d[:],
                    in_offset=IndirectOffsetOnAxis(ap=selidx_t[:, j:j + 1], axis=0),
                )

            # ---- decay + cumprod ----
            dec_t = pp.tile([NCH, JW], f32)
            nc.vector.tensor_scalar(out=dec_t[:], in0=cp_t[:], scalar1=-1.0,
                                    scalar2=1.0, op0=op.mult, op1=op.add)
            z16 = pp.tile([NCH, JW], f32)
            nc.vector.memset(z16[:], 0.0)
            P_t = pp.tile([NCH, JW], f32)
            nc.vector.tensor_tensor_scan(out=P_t[:], data0=dec_t[:], data1=z16[:],
                                         initial=1.0, op0=op.mult, op1=op.add)

            # ---- cross-chunk carry weights W (log-space; depends only on cp,
            # so emitted early: keeps the scalar engine's Ln/Exp ahead of the
            # res-prefetch DMA stream on the same engine -> no queue inversion)
            A_t = pp.tile([NCH, 1], f32)
            nc.vector.tensor_scalar(out=A_t[:], in0=P_t[:, JW - 1:JW],
                                    scalar1=1e-38, scalar2=None, op0=op.max)
            Lc_t = pp.tile([NCH, 1], f32)
            nc.scalar.activation(out=Lc_t[:], in_=A_t[:],
                                 func=mybir.ActivationFunctionType.Ln)
            nc.vector.tensor_scalar(out=Lc_t[:], in0=Lc_t[:], scalar1=-87.0,
                                    scalar2=None, op0=op.max)
            # S[c'] = inclusive prefix of ln A  (as column, per-partition scalar)
            S_ps = psp.tile([128, 1], f32, name="S_ps", tag="S_ps")
            nc.tensor.matmul(out=S_ps[:], lhsT=lti_t[:], rhs=Lc_t[:],
                             start=True, stop=True)
            S_t = pp.tile([128, 1], f32)
            nc.vector.tensor_copy(out=S_t[:], in_=S_ps[:])
            # S2b[c', c] = S[c-1] (strict prefix, broadcast down partitions)
            S2b_ps = psp.tile([128, 128], f32, name="S2b_ps", tag="S2b_ps")
            nc.tensor.matmul(out=S2b_ps[:], lhsT=Lc_t[:].to_broadcast([128, 128]),
                             rhs=lts_t[:], start=True, stop=True)
            # W[c', c] = exp(min(S2[c] - S[c'], mask)) ; mask kills c' >= c
            W_t = pp.tile([128, 128], f32)
            nc.vector.tensor_scalar(out=W_t[:], in0=S2b_ps[:], scalar1=S_t[:, 0:1],
                                    scalar2=None, op0=op.subtract)
            nc.vector.tensor_tensor(out=W_t[:], in0=W_t[:], in1=mb_t[:], op=op.min)
            nc.scalar.activation(out=W_t[:], in_=W_t[:],
                                 func=mybir.ActivationFunctionType.Exp)

            # ---- prefetch residual tiles early (fills DMA idle time in scan).
            # On the scalar engine's HWDGE queue: keeps the slot-limited
            # prefetch stream off the sync queue, which must issue the table
            # write (that the expansion adds transitively depend on) later.
            res_tiles = []
            for t in range(NLT):
                r = resp.tile([128, D], f32, name="r", tag="r")
                nc.scalar.dma_start(out=r[:], in_=res_d[t * 128:(t + 1) * 128, :])
                res_tiles.append(r)

            # ---- b scale (in place): CH_j = cp_j * CH_j ----
            for j in range(JW):
                nc.vector.tensor_scalar(out=CH[:, j * D:(j + 1) * D],
                                        in0=CH[:, j * D:(j + 1) * D],
                                        scalar1=cp_t[:, j:j + 1], scalar2=None,
                                        op0=op.mult)
            # fold initial state into b[0] (chunk 0 only): b0 += dec0 * state
            nc.vector.scalar_tensor_tensor(out=CH[0:1, 0:D], in0=state_t[:],
                                           scalar=dec_t[0:1, 0:1], in1=CH[0:1, 0:D],
                                           op0=op.mult, op1=op.add)

            # ---- pass1: in-chunk scan, in place ----
            for j in range(1, JW):
                nc.vector.scalar_tensor_tensor(
                    out=CH[:, j * D:(j + 1) * D],
                    in0=CH[:, (j - 1) * D:j * D],
                    scalar=dec_t[:, j:j + 1],
                    in1=CH[:, j * D:(j + 1) * D],
                    op0=op.mult, op1=op.add)

            # ---- pass2: carry[c] = sum_{c'<c} W[c',c] * Hend[c']
            carry_ps = psp.tile([128, D], f32, name="carry_ps", tag="carry_ps")
            hend = CH[:, (JW - 1) * D:JW * D]
            for h in range(2):
                nc.tensor.matmul(out=carry_ps[:, h * 512:(h + 1) * 512],
                                 lhsT=W_t[:],
                                 rhs=hend[:, h * 512:(h + 1) * 512],
                                 start=True, stop=True)

            # ---- pass3: E = H + P_j * carry  -> bf16 table tile ----
            E_bf = pp.tile([NCH, JW * D], bf16)
            for j in range(JW):
                nc.vector.scalar_tensor_tensor(
                    out=E_bf[:, j * D:(j + 1) * D],
                    in0=carry_ps[:],
                    scalar=P_t[:, j:j + 1],
                    in1=CH[:, j * D:(j + 1) * D],
                    op0=op.mult, op1=op.add)

            # ---- table -> DRAM ----
            nc.sync.dma_start(
                out=table[0:M, :].rearrange("(c r) d -> c (r d)", c=NCH),
                in_=E_bf[:])
            zrow = pp.tile([1, D], bf16)
            nc.vector.memset(zrow[:], 0.0)
            nc.sync.dma_start(out=table[ZROW:ZROW + 1, :], in_=zrow[:])
            srow = pp.tile([1, D], bf16)
            nc.vector.tensor_copy(out=srow[:], in_=state_t[:])
            nc.sync.dma_start(out=table[SROW:SROW + 1, :], in_=srow[:])

            # ---- expansion: out[l] = res[l] + table[cig[l]] ----
            for t in range(NLT):
                g = gathp.tile([128, D], bf16, name="g", tag="g")
                nc.gpsimd.indirect_dma_start(
                    out=g[:], out_offset=None, in_=table[:],
                    in_offset=IndirectOffsetOnAxis(ap=cig_t[:, t:t + 1], axis=0))
                o = outp.tile([128, D], f32, name="o", tag="o")
                nc.vector.tensor_tensor(out=o[:], in0=res_tiles[t][:], in1=g[:],
                                        op=op.add)
                nc.sync.dma_start(out=out_d[t * 128:(t + 1) * 128, :], in_=o[:])

            # ---- new_state: table[last_idx] -> out row 8192 ----
            g2 = gathp.tile([128, D], bf16, name="g2", tag="g")
            nc.gpsimd.indirect_dma_start(
                out=g2[:], out_offset=None, in_=table[:],
                in_offset=IndirectOffsetOnAxis(ap=cig_t[:, NLT:NLT + 1], axis=0))
            ns = pp.tile([1, D], f32)
            nc.vector.tensor_copy(out=ns[:], in_=g2[0:1, :])
            nc.sync.dma_start(out=out_d[L:L + 1, :], in_=ns[:])

    nc.compile()
    return nc


def _host_prep(prob_row, mask_row, counts_out=None):
    """Per-row index/prob prep. Returns dict of aux input arrays."""
    mask = mask_row.astype(bool)
    counts = int(mask.sum())
    sel = np.argsort(~mask, kind="stable")[:M].astype(np.int32)
    valid = (np.arange(M) < counts)
    cp = (prob_row[sel] * valid).astype(np.float32)
    chunk_idx = np.cumsum(mask.astype(np.int64)) - 1
    ci = np.clip(chunk_idx, 0, M - 1)
    cig = np.where(chunk_idx >= 0, ci, ZROW).astype(np.int32)
    last_idx = (counts - 1) if counts > 0 else SROW
    cig_in = np.concatenate(
        [cig.reshape(NLT, 128).T,
         np.full((128, 1), last_idx, np.int32)], axis=1)
    return {
        "selidx": np.ascontiguousarray(sel.reshape(NCH, JW)),
        "cp": np.ascontiguousarray(cp.reshape(NCH, JW)),
        "cig": np.ascontiguousarray(cig_in),
    }


def kernel(x, residual, prob, token_mask, state):
    from concourse import bass_utils

    if "nc" not in _CACHE:
        _CACHE["nc"] = _build()
    nc = _CACHE["nc"]

    in_maps = []
    for b in range(B):
        aux = _host_prep(np.asarray(prob[b]), np.asarray(token_mask[b]))
        in_maps.append({
            "x": np.ascontiguousarray(x[b], dtype=np.float32),
            "res": np.ascontiguousarray(residual[b], dtype=np.float32),
            "state": np.ascontiguousarray(state[b], dtype=np.float32).reshape(1, D),
            **aux,
        })

    res = bass_utils.run_bass_kernel_spmd(nc, in_maps, core_ids=list(range(B)))
    output = np.stack([res.results[b]["out"][:L] for b in range(B)])
    new_state = np.stack([res.results[b]["out"][L] for b in range(B)])
    return output, new_state


if __name__ == "__main__":
    # quick numpy emulation self-check of the scan decomposition
    rng = np.random.default_rng(0)
    dec = rng.uniform(0.0, 1.0, (NCH, JW)).astype(np.float32)
    b = rng.standard_normal((NCH, JW, 4)).astype(np.float32)
    # flat reference scan
    h = np.zeros(4, np.float64)
    ref = np.zeros((NCH, JW, 4))
    for c in range(NCH):
        for j in range(JW):
            h = dec[c, j] * h + b[c, j]
            ref[c, j] = h
    # chunked
    H = np.zeros_like(b)
    acc = np.zeros((NCH, 4))
    P = np.cumprod(dec, axis=1)
    for j in range(JW):
        acc = dec[:, j:j + 1] * acc + b[:, j]
        H[:, j] = acc
    A = np.maximum(P[:, -1], 1e-38)
    Lg = np.maximum(np.log(A), -87)
    S = np.cumsum(Lg)
    S2 = S - Lg  # strict prefix
    kk = np.arange(NCH)[:, None]
    mm = np.arange(NCH)[None, :]
    W = np.exp(np.minimum(S2[None, :] - S[:, None], np.where(kk < mm, 0.0, -1e5)))
    carry = W.T @ H[:, -1]
    E = H + P[:, :, None] * carry[:, None, :]
    print("scan decomposition max err:", np.abs(E - ref).max())
